# revision 8
# baseline (speedup 1.0000x reference)
"""AFT-Full (Attention-Free Transformer) distributed Bass kernel for 8 TRN2 NeuronCores.

Reference math (B=4, N=512, D=128):
    q = sigmoid(x @ Wq^T + bq); k = x @ Wk^T + bk; v = x @ Wv^T + bv
    s[b,t,j,d] = k[b,j,d] + pos_bias[t,j];  m = max_j s
    out = q * (sum_j exp(s-m) * v) / (sum_j exp(s-m))

The max-stabilizer m cancels between numerator and denominator, and
exp(k + pb) = exp(pb) * exp(k), so with P = exp(pos_bias), ek = exp(k):
    out = q * (P @ (ek * v)) / (P @ ek)        (matmuls contract over j)
Further, sigmoid(q)/den = 1/(den * (1 + exp(-qlin))), so the epilogue
needs only Exp activations (one ACT table) and one fast reciprocal:
    out^T = num^T * recip(den^T * (1 + exp(-qlin^T)))

Sharding: 8 cores = 4 batches x 2 t-halves; no collectives. Each core gets
x[b]^T with its t-half's columns rotated to the front, and pos_bias rows
rotated identically, so the j-contraction order matches and one SPMD graph
serves all cores. Device computes out^T[d, t] for its (b, t-half).

Compute dtype: bf16 into the PE array (fp32 PSUM accumulation) — fp32
matmuls run ~4x slower on trn2. f32->bf16 conversion happens inside the
SWDGE cast-DMAs on the otherwise-idle GpSimd engine. Biases are folded in
as rank-1 matmuls accumulating into the same PSUM bank (free on the PE).
"""

import sys

import numpy as np

try:
    import concourse.bass as bass
except ImportError:  # pragma: no cover
    sys.path.insert(0, "/opt/trn_rl_repo")
    import concourse.bass as bass

import concourse.mybir as mybir
import concourse.tile as tile
from concourse import bacc
from concourse.bass_utils import run_bass_kernel_spmd

F32 = mybir.dt.float32
BF16 = mybir.dt.bfloat16
B, N, D = 4, 512, 128
T = N // 2  # t-rows per core
JT = N // 128  # j tiles of 128
AF = mybir.ActivationFunctionType


def build_nc() -> bass.Bass:
    # Bacc (not plain Bass): its compile() pass legalizes multi-wait
    # instructions (move_matmul_waits_to_ldweights, event semaphores),
    # which this walrus build requires.
    nc = bacc.Bacc()
    # x0|x1 = halves of x[b]^T; w = [Wq^T | Wk^T | Wv^T]
    x0 = nc.dram_tensor("x0", [D, N // 2], F32, kind="ExternalInput")
    x1 = nc.dram_tensor("x1", [D, N // 2], F32, kind="ExternalInput")
    wall = nc.dram_tensor("wall", [D, 3 * D], F32, kind="ExternalInput")
    ball = nc.dram_tensor("ball", [1, 3 * D], F32, kind="ExternalInput")  # bq|bk|bv
    # pos_bias^T packed so each partition's 4 j-tiles are contiguous
    pbT = nc.dram_tensor("pbT", [128, JT * T], F32, kind="ExternalInput")
    out = nc.dram_tensor("out", [D, T], F32, kind="ExternalOutput")

    with tile.TileContext(nc) as tc:
        with (
            tc.tile_pool(name="sb", bufs=1) as sb,
            tc.tile_pool(name="ps", bufs=1, space="PSUM") as ps,
        ):
            # ---- loads: staged SWDGE cast-DMAs f32->bf16 on GpSimd ----
            bb = sb.tile([1, 3 * D], BF16, name="bb")
            nc.gpsimd.dma_start(bb[:], ball[:])
            wb = sb.tile([D, 3 * D], BF16, name="wb")
            nc.gpsimd.dma_start(wb[:], wall[:])
            x0b = sb.tile([D, N // 2], BF16, name="x0b")
            nc.gpsimd.dma_start(x0b[:], x0[:])
            x1b = sb.tile([D, N // 2], BF16, name="x1b")
            nc.gpsimd.dma_start(x1b[:], x1[:])
            # pos_bias^T: contiguous rows, HWDGE on the otherwise-free SP ring
            pb_sb = sb.tile([128, JT, T], F32, name="pb_sb")
            nc.sync.dma_start(pb_sb[:], pbT[:].rearrange("p (j t) -> p j t", t=T))

            ones = sb.tile([1, T], BF16, name="ones")
            nc.vector.memset(ones[:], 1.0)

            # ---- P^T = exp(pos_bias^T), one big ACT op -> bf16 ----
            pt = sb.tile([128, JT, T], BF16, name="pt")
            nc.scalar.activation(pt[:], pb_sb[:], AF.Exp)

            # ---- k/v projections: kv_all[:, j, 0:128]=k_j+bk, [...,128:256]=v_j+bv ----
            kv_all = ps.tile([128, JT, 2 * D], F32, tag="kv_all")
            xh = [x0b, x0b, x1b, x1b]

            def kv_pair(j):
                nc.tensor.matmul(
                    kv_all[:, j, :], ones[:, 0:128], bb[:, D : 3 * D],
                    start=True, stop=False,
                )
                nc.tensor.matmul(
                    kv_all[:, j, :],
                    xh[j][:, (j % 2) * 128 : (j % 2) * 128 + 128],
                    wb[:, D : 3 * D],
                    start=False, stop=True,
                )

            kv_pair(0)
            kv_pair(1)
            # q^T[d,t] = bq + Wq @ x[t-half]^T  (needs only x0)
            q_ps = ps.tile([D, T], F32, tag="q_ps")
            nc.tensor.matmul(q_ps[:], bb[:, 0:D], ones[:], start=True, stop=False)
            nc.tensor.matmul(q_ps[:], wb[:, 0:D], x0b[:], start=False, stop=True)
            kv_pair(2)
            kv_pair(3)

            # ---- exp(k) and ek*v, split in j-halves to unblock den earlier ----
            ek = sb.tile([128, JT, D], BF16, name="ek")
            wt = sb.tile([128, JT, D], BF16, name="wt")
            nc.scalar.activation(ek[:, 0:2, :], kv_all[:, 0:2, 0:D], AF.Exp)
            nc.vector.tensor_mul(wt[:, 0:2, :], ek[:, 0:2, :], kv_all[:, 0:2, D : 2 * D])
            nc.scalar.activation(ek[:, 2:4, :], kv_all[:, 2:4, 0:D], AF.Exp)
            nc.vector.tensor_mul(wt[:, 2:4, :], ek[:, 2:4, :], kv_all[:, 2:4, D : 2 * D])
            # exp(-qlin), then g = 1 + exp(-qlin)  (off the critical tail)
            eq = sb.tile([D, T], F32, name="eq")
            nc.scalar.activation(eq[:], q_ps[:], AF.Exp, scale=-1.0)
            g = sb.tile([D, T], F32, name="g")
            nc.vector.tensor_scalar_add(g[:], eq[:], 1.0)

            # ---- den^T = sum_j ek_j @ pt_j ; num^T = sum_j wt_j @ pt_j ----
            den_ps = ps.tile([D, T], F32, tag="den_ps")
            num_ps = ps.tile([D, T], F32, tag="num_ps")
            for j in range(JT):
                nc.tensor.matmul(
                    den_ps[:], ek[:, j, :], pt[:, j, :],
                    start=(j == 0), stop=(j == JT - 1),
                )
            for j in range(JT):
                nc.tensor.matmul(
                    num_ps[:], wt[:, j, :], pt[:, j, :],
                    start=(j == 0), stop=(j == JT - 1),
                )

            # ---- out^T = num^T * recip(den^T * g), halved to overlap DMA-out ----
            f = sb.tile([D, T], F32, name="f")
            nc.vector.tensor_mul(f[:], g[:], den_ps[:])
            rec = sb.tile([D, T], F32, name="rec")
            nc.vector.reciprocal_approx_fast(rec[:], f[:])
            out_sb = sb.tile([D, T], F32, name="out_sb")
            half = T // 2
            nc.vector.tensor_mul(
                out_sb[:, 0:half], rec[:, 0:half], num_ps[:, 0:half]
            )
            nc.sync.dma_start(out[:, 0:half], out_sb[:, 0:half])
            nc.vector.tensor_mul(
                out_sb[:, half:T], rec[:, half:T], num_ps[:, half:T]
            )
            nc.sync.dma_start(out[:, half:T], out_sb[:, half:T])

    nc.finalize()
    return nc


def prepare_in_maps(x, Wq, bq, Wk, bk, Wv, bv, pos_bias):
    x = np.asarray(x, dtype=np.float32)
    pos_bias = np.asarray(pos_bias, dtype=np.float32)
    wall = np.concatenate(
        [
            np.asarray(Wq, np.float32).T,
            np.asarray(Wk, np.float32).T,
            np.asarray(Wv, np.float32).T,
        ],
        axis=1,
    )
    ball = np.ascontiguousarray(
        np.concatenate(
            [np.asarray(bq, np.float32), np.asarray(bk, np.float32),
             np.asarray(bv, np.float32)]
        )[None]
    )

    wall = np.ascontiguousarray(wall)
    in_maps = []
    for i in range(8):
        b, th = divmod(i, 2)
        t0 = th * T
        perm = np.concatenate([np.arange(t0, N), np.arange(0, t0)])
        xT = x[b][perm].T  # [128, 512]
        pb = pos_bias[t0 : t0 + T][:, perm].T  # [512, 256] (j, t)
        # pack so each SBUF partition's 4 j-tiles are contiguous: [128, 4*256]
        pb2 = np.ascontiguousarray(
            pb.reshape(JT, 128, T).transpose(1, 0, 2).reshape(128, JT * T)
        )
        in_maps.append(
            {
                "x0": np.ascontiguousarray(xT[:, 0 : N // 2]),
                "x1": np.ascontiguousarray(xT[:, N // 2 : N]),
                "wall": wall,
                "ball": ball,
                "pbT": pb2,
            }
        )
    return in_maps


def assemble_output(results) -> np.ndarray:
    out = np.empty((B, N, D), np.float32)
    for i in range(8):
        b, th = divmod(i, 2)
        t0 = th * T
        out[b, t0 : t0 + T, :] = results[i]["out"].T
    return out


def kernel(x, Wq, bq, Wk, bk, Wv, bv, pos_bias) -> np.ndarray:
    in_maps = prepare_in_maps(x, Wq, bq, Wk, bk, Wv, bv, pos_bias)
    nc = build_nc()
    res = run_bass_kernel_spmd(nc, in_maps, core_ids=list(range(8))).results
    return assemble_output(res)


if __name__ == "__main__":
    rng = np.random.default_rng(0)
    s = 1.0 / np.sqrt(D)
    inputs = dict(
        x=rng.standard_normal((B, N, D), dtype=np.float32),
        Wq=rng.standard_normal((D, D), dtype=np.float32) * s,
        bq=rng.standard_normal((D,), dtype=np.float32) * s,
        Wk=rng.standard_normal((D, D), dtype=np.float32) * s,
        bk=rng.standard_normal((D,), dtype=np.float32) * s,
        Wv=rng.standard_normal((D, D), dtype=np.float32) * s,
        bv=rng.standard_normal((D,), dtype=np.float32) * s,
        pos_bias=rng.standard_normal((N, N), dtype=np.float32) * 0.1,
    )
    out = kernel(**inputs)
    print("kernel ran, out shape:", out.shape)
